# revision 18
# baseline (speedup 1.0000x reference)
"""Trainium2 Bass kernel for nn_patch_expanding.

Computes, for x [32, 1024, 1024] and w [512, 512]:
    xg = x.reshape(B, 32, 32, 1024); x0, x1 = split(xg, channel halves)
    xi = row-interleave(x0, x1) -> [B, 64, 32, 512]
    y  = xi @ w -> reshape [B, 2048, 512]

Strategy: data-parallel over batch (4 batches/core on 8 cores). Per core the
op is a [4096, 1024] -> [8192, 512] GEMM (contraction over cin=512 per output
row, both channel halves sharing w) plus a row permutation that is folded into
the PSUM-eviction access pattern. The contraction must sit on SBUF partitions,
so x tiles are transposed on the tensor engine (fp32 transpose mode), rounded
to fp32r during PSUM eviction on DVE, and fed as stationary operands to
full-rate fp32r matmuls with w moving (N=512). Loads/stores are batched 4
tiles (2 MB) per DMA; ACT evicts matmul PSUM two banks at a time and issues
the y stores itself.
"""
import sys
sys.path.insert(0, "/opt/trn_rl_repo")
import numpy as np

B, L, C = 32, 1024, 1024
NCORES = 8
BPC = B // NCORES          # batches per core
ROWS = BPC * L             # 4096 x-rows per core
OROWS = 2 * ROWS           # 8192 y-rows per core
NDB = ROWS // 128          # 32 pipeline tiles per core
G = 4                      # tiles per DMA group (2 MB loads / stores)
NG = NDB // G

_CACHE = {}


def _build(reps: int = 1):
    import concourse.bass as bass
    from concourse import mybir

    f32, f32r = mybir.dt.float32, mybir.dt.float32r
    nc = bass.Bass(trn_type="TRN2", target_bir_lowering=False, debug=False,
                   num_devices=NCORES)

    xd = nc.dram_tensor("x", [ROWS, C], f32, kind="ExternalInput").ap()
    wd = nc.dram_tensor("w", [512, 512], f32, kind="ExternalInput").ap()
    yd = nc.dram_tensor("y", [OROWS, 512], f32, kind="ExternalOutput").ap()

    # NOTE: completion increments of concurrently in-flight DMAs interleave on
    # a shared sem (16 per-engine +1s each), so load/store sems are split by
    # buffer parity: a threshold then implies one specific transfer completed.
    s_ld = [nc.alloc_semaphore("s_ld0"), nc.alloc_semaphore("s_ld1")]
    s_lw = nc.alloc_semaphore("s_lw")    # w load
    s_tr = nc.alloc_semaphore("s_tr")    # PE transposes done, +1 per tile
    s_xt = nc.alloc_semaphore("s_xt")    # DVE xt evictions done, +1 per tile
    s_mm = nc.alloc_semaphore("s_mm")    # PE matmuls done, +1 per tile
    s_ye = nc.alloc_semaphore("s_ye")    # ACT psum evictions done, +1 per tile
    s_st = [nc.alloc_semaphore("s_st0"), nc.alloc_semaphore("s_st1")]
    s_w = nc.alloc_semaphore("s_w")      # w rounded to fp32r
    s_id = nc.alloc_semaphore("s_id")    # identity ready
    all_sems = s_ld + s_st + [s_lw, s_tr, s_xt, s_mm, s_ye, s_w, s_id]

    T = NDB * reps

    with (
        nc.sbuf_tensor("xin", [128, 2, G, 1024], f32) as xin,
        nc.sbuf_tensor("xt", [128, 2, 4, 256], f32) as xt,
        nc.sbuf_tensor("wsb", [128, 4, 512], f32) as wsb,
        nc.sbuf_tensor("wr", [128, 4, 512], f32) as wr,
        nc.sbuf_tensor("yo", [128, 2, 2 * G, 512], f32) as yo,
        nc.sbuf_tensor("ident", [128, 128], f32) as ident,
        nc.psum_tensor("tp", [128, 4, 512], f32) as tp,
        nc.psum_tensor("mm", [128, 4, 512], f32) as mm,
    ):
        xin_a, xt_a, wsb_a, wr_a = xin.ap(), xt.ap(), wsb.ap(), wr.ap()
        yo_a, id_a, tp_a, mm_a = yo.ap(), ident.ap(), tp.ap(), mm.ap()
        # xt viewed with the (d, s, w32) row split used by the evict scatter
        xt_v = xt_a.rearrange("p par kk (d s2 q) -> p par kk d s2 q", d=4, s2=2, q=32)

        # sems are NOT guaranteed zero at kernel entry (device state persists
        # across executions and barriers are unreliable in this runtime), so no
        # engine may trust a wait before the sems are cleared. gpsimd clears
        # them immediately at start while every other engine sits in a dead
        # -wait long enough (~20us) to guarantee the clears landed.
        for s in all_sems:
            nc.gpsimd.sem_clear(s)
        for eng in (nc.sync, nc.tensor, nc.vector, nc.scalar):
            for _ in range(4):
                eng.nop(cycle_cnt=6000, nofuse=True)

        with nc.Block() as block:

            @block.gpsimd
            def _(g):
                g.memset(id_a[:], 0.0)
                g.affine_select(
                    out=id_a[:], in_=id_a[:],
                    compare_op=mybir.AluOpType.not_equal,
                    fill=1.0, base=0,
                    pattern=[[-1, 128]], channel_multiplier=1,
                )
                g.drain().then_inc(s_id)
                # do not let the program end before the last store lands, and
                # leave the sems clean for the next execution
                g.wait_ge(s_st[0], 16 * (NG * reps // 2))
                g.wait_ge(s_st[1], 16 * (NG * reps // 2))
                for s in all_sems:
                    g.sem_clear(s)

            @block.sync
            def _(sp):
                # w first, then x group loads (2 MB each, double-buffered)
                sp.dma_start(wsb_a[:], wd.rearrange("(kk p) n -> p kk n", p=128)
                             ).then_inc(s_lw, 16)
                for gg in range(NG * reps):
                    gpar, ga = gg % 2, gg % NG
                    if gg >= 2:
                        sp.wait_ge(s_tr, 4 * gg - 4)      # xin[gpar] free
                    sp.dma_start(
                        xin_a[:, gpar, :, :],
                        xd[512 * ga:512 * ga + 512, :].rearrange(
                            "(o p) c -> p o c", p=128),
                    ).then_inc(s_ld[gpar], 16)

            @block.tensor
            def _(pe):
                pe.wait_ge(s_id, 1)
                pe.wait_ge(s_w, 1)
                for it in range(T + 1):
                    if it < T:
                        t, par = it, it % 2
                        gg, o = t // G, t % G
                        gpar = gg % 2
                        if o == 0:
                            pe.wait_ge(s_ld[gpar], 16 * (gg // 2 + 1))
                        # tp[par] free: covered by MM(it-2)'s s_xt wait
                        for s in (0, 1):
                            for kk in range(4):
                                inst = pe.matmul(
                                    tp_a[:, 2 * par + s, 128 * kk:128 * kk + 128],
                                    xin_a[:, gpar, o, 512 * s + 128 * kk:512 * s + 128 * kk + 128],
                                    id_a[:],
                                    is_transpose=True,
                                    start=(kk == 0), stop=(kk == 3),
                                )
                                if (s, kk) == (1, 3):
                                    inst.then_inc(s_tr)
                    if it >= 1:
                        t, par = it - 1, (it - 1) % 2
                        pe.wait_ge(s_xt, t + 1)           # xt[par] ready
                        if t >= 2:
                            pe.wait_ge(s_ye, t - 1)       # mm[par] free
                        for blk in (0, 1):
                            for kk in range(4):
                                inst = pe.matmul(
                                    mm_a[:, 2 * par + blk, :],
                                    xt_a[:, par, kk, 128 * blk:128 * blk + 128].bitcast(f32r),
                                    wr_a[:, kk, :].bitcast(f32r),
                                    start=(kk == 0), stop=(kk == 3),
                                )
                                if (blk, kk) == (1, 3):
                                    inst.then_inc(s_mm)

            @block.vector
            def _(dv):
                dv.wait_ge(s_lw, 16)
                dv.tensor_copy(wr_a[:].bitcast(f32r), wsb_a[:])
                dv.drain().then_inc(s_w)
                for t in range(T):
                    par = t % 2
                    dv.wait_ge(s_tr, t + 1)               # tp[par] filled
                    if t >= 2:
                        dv.wait_ge(s_mm, t - 1)           # xt[par] free
                    for s in (0, 1):
                        dv.tensor_copy(
                            xt_v[:, par, :, :, s, :].bitcast(f32r),
                            tp_a[:, 2 * par + s, :].rearrange(
                                "p (kk d q) -> p kk d q", kk=4, d=4, q=32),
                        )
                    dv.drain().then_inc(s_xt)

            @block.scalar
            def _(ac):
                for t in range(T):
                    par = t % 2
                    gg, o = t // G, t % G
                    gpar, ga = gg % 2, (t % NDB) // G
                    if o == 0 and gg >= 2:
                        ac.wait_ge(s_st[gpar], 16 * (gg // 2))   # yo[gpar] free
                    ac.wait_ge(s_mm, t + 1)               # mm[par] filled
                    ac.copy(yo_a[:, gpar, 2 * o:2 * o + 2, :],
                            mm_a[:, 2 * par:2 * par + 2, :]).then_inc(s_ye)
                    if o == G - 1:
                        ac.drain()
                        ac.dma_start(
                            yd[1024 * ga:1024 * ga + 1024, :].rearrange(
                                "(o p) n -> p o n", p=128),
                            yo_a[:, gpar, :, :],
                        ).then_inc(s_st[gpar], 16)

    return nc


def kernel(x: np.ndarray, w: np.ndarray) -> np.ndarray:
    from concourse.bass_utils import run_bass_kernel_spmd

    if "nc" not in _CACHE:
        _CACHE["nc"] = _build()
    nc = _CACHE["nc"]

    x = np.ascontiguousarray(x, dtype=np.float32)
    w = np.ascontiguousarray(w, dtype=np.float32)
    xs = x.reshape(NCORES, ROWS, C)
    in_maps = [{"x": xs[i], "w": w} for i in range(NCORES)]
    res = run_bass_kernel_spmd(nc, in_maps, list(range(NCORES)))
    y = np.stack([res.results[i]["y"] for i in range(NCORES)], axis=0)
    return y.reshape(B, 2 * L, C // 2)


# revision 19
# speedup vs baseline: 1.2003x; 1.2003x over previous
"""Trainium2 Bass kernel for nn_patch_expanding.

Computes, for x [32, 1024, 1024] and w [512, 512]:
    xg = x.reshape(B, 32, 32, 1024); x0, x1 = split(xg, channel halves)
    xi = row-interleave(x0, x1) -> [B, 64, 32, 512]
    y  = xi @ w -> reshape [B, 2048, 512]

Strategy: data-parallel over batch (4 batches/core on 8 cores). Per core the
op is a [4096, 1024] -> [8192, 512] GEMM (contraction over cin=512 per output
row, both channel halves sharing w) plus a row permutation that is folded into
the PSUM-eviction access pattern. The contraction must sit on SBUF partitions,
so x tiles are transposed on the tensor engine (fp32 transpose mode), rounded
to fp32r during PSUM eviction on DVE, and fed as stationary operands to
full-rate fp32r matmuls with w moving (N=512). Loads/stores are batched 4
tiles (2 MB) per DMA; ACT evicts matmul PSUM two banks at a time and issues
the y stores itself.
"""
import sys
sys.path.insert(0, "/opt/trn_rl_repo")
import numpy as np

B, L, C = 32, 1024, 1024
NCORES = 8
BPC = B // NCORES          # batches per core
ROWS = BPC * L             # 4096 x-rows per core
OROWS = 2 * ROWS           # 8192 y-rows per core
NDB = ROWS // 128          # 32 pipeline tiles per core
G = 4                      # tiles per DMA group (2 MB loads / stores)
NG = NDB // G

_CACHE = {}


def _build(reps: int = 1):
    import concourse.bass as bass
    from concourse import mybir

    f32, f32r = mybir.dt.float32, mybir.dt.float32r
    nc = bass.Bass(trn_type="TRN2", target_bir_lowering=False, debug=False,
                   num_devices=NCORES)

    xd = nc.dram_tensor("x", [ROWS, C], f32, kind="ExternalInput").ap()
    wd = nc.dram_tensor("w", [512, 512], f32, kind="ExternalInput").ap()
    yd = nc.dram_tensor("y", [OROWS, 512], f32, kind="ExternalOutput").ap()

    # NOTE: completion increments of concurrently in-flight DMAs interleave on
    # a shared sem (16 per-engine +1s each), so load/store sems are split by
    # buffer parity: a threshold then implies one specific transfer completed.
    s_ld = [nc.alloc_semaphore("s_ld0"), nc.alloc_semaphore("s_ld1")]
    s_lw = nc.alloc_semaphore("s_lw")    # w load
    s_tr = nc.alloc_semaphore("s_tr")    # PE transposes done, +1 per tile
    s_xt = nc.alloc_semaphore("s_xt")    # DVE xt evictions done, +1 per tile
    s_mm = nc.alloc_semaphore("s_mm")    # PE matmuls done, +1 per tile
    s_ye = nc.alloc_semaphore("s_ye")    # ACT psum evictions done, +1 per tile
    s_st = [nc.alloc_semaphore("s_st0"), nc.alloc_semaphore("s_st1")]
    s_w = nc.alloc_semaphore("s_w")      # w rounded to fp32r
    s_id = nc.alloc_semaphore("s_id")    # identity ready
    all_sems = s_ld + s_st + [s_lw, s_tr, s_xt, s_mm, s_ye, s_w, s_id]

    T = NDB * reps

    with (
        nc.sbuf_tensor("xin", [128, 2, G, 1024], f32) as xin,
        nc.sbuf_tensor("xt", [128, 2, 4, 256], f32) as xt,
        nc.sbuf_tensor("wsb", [128, 4, 512], f32) as wsb,
        nc.sbuf_tensor("wr", [128, 4, 512], f32) as wr,
        nc.sbuf_tensor("yo", [128, 2, 2 * G, 512], f32) as yo,
        nc.sbuf_tensor("ident", [128, 128], f32) as ident,
        nc.psum_tensor("tp", [128, 4, 512], f32) as tp,
        nc.psum_tensor("mm", [128, 4, 512], f32) as mm,
    ):
        xin_a, xt_a, wsb_a, wr_a = xin.ap(), xt.ap(), wsb.ap(), wr.ap()
        yo_a, id_a, tp_a, mm_a = yo.ap(), ident.ap(), tp.ap(), mm.ap()
        # xt viewed with the (d, s, w32) row split used by the evict scatter
        xt_v = xt_a.rearrange("p par kk (d s2 q) -> p par kk d s2 q", d=4, s2=2, q=32)

        # sems are NOT guaranteed zero at kernel entry (device state persists
        # across executions and barriers are unreliable in this runtime), so no
        # engine may trust a wait before the sems are cleared. gpsimd clears
        # them immediately at start while every other engine sits in a dead
        # -wait long enough (~20us) to guarantee the clears landed.
        for s in all_sems:
            nc.gpsimd.sem_clear(s)
        for eng in (nc.sync, nc.tensor, nc.vector, nc.scalar):
            for _ in range(4):
                eng.nop(cycle_cnt=6000, nofuse=True)

        with nc.Block() as block:

            @block.gpsimd
            def _(g):
                g.memset(id_a[:], 0.0)
                g.affine_select(
                    out=id_a[:], in_=id_a[:],
                    compare_op=mybir.AluOpType.not_equal,
                    fill=1.0, base=0,
                    pattern=[[-1, 128]], channel_multiplier=1,
                )
                g.drain().then_inc(s_id)
                # do not let the program end before the last store lands, and
                # leave the sems clean for the next execution
                g.wait_ge(s_st[0], 16 * (NG * reps // 2))
                g.wait_ge(s_st[1], 16 * (NG * reps // 2))
                for s in all_sems:
                    g.sem_clear(s)

            @block.sync
            def _(sp):
                # w first, then x group loads (2 MB each, double-buffered)
                sp.dma_start(wsb_a[:], wd.rearrange("(kk p) n -> p kk n", p=128)
                             ).then_inc(s_lw, 16)
                for gg in range(NG * reps):
                    gpar, ga = gg % 2, gg % NG
                    if gg >= 2:
                        sp.wait_ge(s_tr, 4 * gg - 4)      # xin[gpar] free
                    sp.dma_start(
                        xin_a[:, gpar, :, :],
                        xd[512 * ga:512 * ga + 512, :].rearrange(
                            "(o p) c -> p o c", p=128),
                    ).then_inc(s_ld[gpar], 16)

            @block.tensor
            def _(pe):
                pe.wait_ge(s_id, 1)
                pe.wait_ge(s_w, 1)
                for it in range(T + 1):
                    if it < T:
                        t, par = it, it % 2
                        gg, o = t // G, t % G
                        gpar = gg % 2
                        if o == 0:
                            pe.wait_ge(s_ld[gpar], 16 * (gg // 2 + 1))
                        # tp[par] free: covered by MM(it-2)'s s_xt wait
                        for s in (0, 1):
                            for kk in range(4):
                                inst = pe.matmul(
                                    tp_a[:, 2 * par + s, 128 * kk:128 * kk + 128],
                                    xin_a[:, gpar, o, 512 * s + 128 * kk:512 * s + 128 * kk + 128],
                                    id_a[:],
                                    is_transpose=True,
                                    start=(kk == 0), stop=(kk == 3),
                                )
                                if (s, kk) == (1, 3):
                                    inst.then_inc(s_tr)
                    if it >= 1:
                        t, par = it - 1, (it - 1) % 2
                        pe.wait_ge(s_xt, t + 1)           # xt[par] ready
                        if t >= 2:
                            pe.wait_ge(s_ye, t - 1)       # mm[par] free
                        for blk in (0, 1):
                            for kk in range(4):
                                inst = pe.matmul(
                                    mm_a[:, 2 * par + blk, :],
                                    xt_a[:, par, kk, 128 * blk:128 * blk + 128].bitcast(f32r),
                                    wr_a[:, kk, :].bitcast(f32r),
                                    start=(kk == 0), stop=(kk == 3),
                                )
                                if (blk, kk) == (1, 3):
                                    inst.then_inc(s_mm)

            @block.vector
            def _(dv):
                dv.wait_ge(s_lw, 16)
                dv.tensor_copy(wr_a[:].bitcast(f32r), wsb_a[:])
                dv.drain().then_inc(s_w)
                for t in range(T):
                    par = t % 2
                    dv.wait_ge(s_tr, t + 1)               # tp[par] filled
                    if t >= 2:
                        dv.wait_ge(s_mm, t - 1)           # xt[par] free
                    dv.tensor_copy(
                        xt_v[:, par].transpose([0, 3, 1, 2, 4]).bitcast(f32r),
                        tp_a[:, 2 * par:2 * par + 2, :].rearrange(
                            "p s2 (kk d q) -> p s2 kk d q", kk=4, d=4, q=32),
                    )
                    dv.drain().then_inc(s_xt)

            @block.scalar
            def _(ac):
                for t in range(T):
                    par = t % 2
                    gg, o = t // G, t % G
                    gpar, ga = gg % 2, (t % NDB) // G
                    if o == 0 and gg >= 2:
                        ac.wait_ge(s_st[gpar], 16 * (gg // 2))   # yo[gpar] free
                    ac.wait_ge(s_mm, t + 1)               # mm[par] filled
                    ac.copy(yo_a[:, gpar, 2 * o:2 * o + 2, :],
                            mm_a[:, 2 * par:2 * par + 2, :]).then_inc(s_ye)
                    if o == G - 1:
                        ac.drain()
                        ac.dma_start(
                            yd[1024 * ga:1024 * ga + 1024, :].rearrange(
                                "(o p) n -> p o n", p=128),
                            yo_a[:, gpar, :, :],
                        ).then_inc(s_st[gpar], 16)

    return nc


def kernel(x: np.ndarray, w: np.ndarray) -> np.ndarray:
    from concourse.bass_utils import run_bass_kernel_spmd

    if "nc" not in _CACHE:
        _CACHE["nc"] = _build()
    nc = _CACHE["nc"]

    x = np.ascontiguousarray(x, dtype=np.float32)
    w = np.ascontiguousarray(w, dtype=np.float32)
    xs = x.reshape(NCORES, ROWS, C)
    in_maps = [{"x": xs[i], "w": w} for i in range(NCORES)]
    res = run_bass_kernel_spmd(nc, in_maps, list(range(NCORES)))
    y = np.stack([res.results[i]["y"] for i in range(NCORES)], axis=0)
    return y.reshape(B, 2 * L, C // 2)
